# revision 15
# baseline (speedup 1.0000x reference)
"""LoRA attention kernel for Trainium2, batch-sharded across 8 NeuronCores.

Strategy (v4):
  - Data parallel: batch B=8 -> one batch element per core.
  - LoRA factors and the 1/sqrt(hd) score scale are folded into Wqkv on the
    host (exact algebra, float64).
  - All matmul operands are bf16 (PSUM accumulation stays fp32): halves SBUF
    traffic and enables FWL fast weight loads.
  - q,k are produced transposed ([head_dim, tokens]) directly from x^T so the
    score matmuls need no on-chip transposes. v is produced in natural layout
    with an extra all-ones column per head, so the attention-value matmul
    accumulates the softmax denominators for free in row 64 of its output.
  - Score matmuls run K=64 as row-tiled pairs: even key-tiles on PE rows
    0-63 (tile_position (0,0)), odd key-tiles on rows 64-127 ((64,0)).
    Adjacent issue makes each (even,odd) pair execute concurrently in
    disjoint array row-groups (~310ns/pair vs 430ns serial).
  - Scores land in [128, 1024] PSUM supertiles (2 banks, 3 rotating bufs);
    one ACTIVATE(Exp) covers 2 key-tiles, amortizing ScalarE's ~352-cycle
    per-instruction overhead. ScalarE is the pacing engine: the whole kernel
    is software-pipelined so exp inputs are always ready ahead of it --
    score groups of chunk s are emitted while attn*v groups run 1-3 chunks
    behind (v-aug construction fills the early-chunk PE slack).
"""
import numpy as np
import ml_dtypes

import concourse.bass as bass
import concourse.bacc as bacc
import concourse.mybir as mybir
import concourse.tile as tile
from concourse.bass_utils import run_bass_kernel_spmd

F32 = mybir.dt.float32
BF16 = mybir.dt.bfloat16
EXP = mybir.ActivationFunctionType.Exp

B, N, C, H, HD = 8, 1024, 768, 12, 64
CT = C // 128           # 6 contraction tiles over C
QC = N // 512           # 2 query chunks of 512
KT = N // 128           # 8 key tiles of 128
SCALE = HD ** -0.5
N_CORES = 8
VW = (H - 1) * 65 + 128  # vaug tile width (65-pitch heads, widened last read)

_NC_CACHE = None


def _build():
    nc = bacc.Bacc(None, target_bir_lowering=False)

    xT = nc.dram_tensor("xT", [C, N], BF16, kind="ExternalInput")
    wqk = nc.dram_tensor("wqk", [H, 128, C], BF16, kind="ExternalInput")
    wv = nc.dram_tensor("wv", [CT, 128, C], BF16, kind="ExternalInput")
    wpt = nc.dram_tensor("wpt", [CT, 128, C], BF16, kind="ExternalInput")
    bias = nc.dram_tensor("bias", [1, C], F32, kind="ExternalInput")
    y = nc.dram_tensor("y", [N, C], F32, kind="ExternalOutput")

    from contextlib import ExitStack
    with tile.TileContext(nc) as tc:
        with ExitStack() as ctx:
            pool = lambda name, bufs, **kw: ctx.enter_context(
                tc.tile_pool(name=name, bufs=bufs, **kw))
            xt_pool = pool("xt", CT)
            wqk_pool = pool("wqkp", 3)
            w768_pool = pool("w768", 2 * CT)      # wv + wpt
            vaug_pool = pool("vaug", KT)
            st_pool = pool("stp", 6)
            ktq_pool = pool("ktq", 8)
            et_pool = pool("etp", 16)
            avs_pool = pool("avsp", 3)
            iv_pool = pool("ivp", 4)
            bc_pool = pool("bcp", 3)
            ost_pool = pool("ostp", 3)
            out_pool = pool("outp", CT)
            y_pool = pool("yp", 3)
            cst_pool = pool("cst", 1)
            sc_ps = pool("sc_ps", 3, space="PSUM")    # [128,1024] supertiles
            av_ps = pool("av_ps", 1, space="PSUM")
            qk_ps = pool("qk_ps", 1, space="PSUM")

            # ---- PE warm-up: bridge the DMA lead-in so the HAM clock gate
            # opens before real work arrives ---------------------------------
            wur = cst_pool.tile([128, 512], BF16, tag="wur")
            nc.vector.memset(wur, 0.0)
            for i in range(8):
                wps = qk_ps.tile([128, 512], F32, tag="qk", name=f"wu{i}")
                nc.tensor.matmul(wps, wur[:, 0:128], wur,
                                 start=True, stop=True)

            # ---- loads -----------------------------------------------------
            def load_wqk(h):
                wt = wqk_pool.tile([128, C], BF16, tag="wqk", name=f"wqk{h}")
                nc.sync.dma_start(out=wt, in_=wqk[h, :, :])
                return wt

            wts0 = load_wqk(0)

            # x tiles via the Activation queue: parallel to the sync-queue
            # weight loads, and ScalarE is idle during the lead-in anyway
            xt = []
            for c in range(CT):
                t = xt_pool.tile([128, N], BF16, tag="xt", name=f"xt{c}")
                nc.scalar.dma_start(out=t, in_=xT[c * 128:(c + 1) * 128, :])
                xt.append(t)

            bias_bc = cst_pool.tile([128, C], F32, tag="biasbc")
            nc.sync.dma_start(out=bias_bc, in_=bias[:, :].to_broadcast([128, C]))
            ones12 = cst_pool.tile([128, H], BF16, tag="ones12")
            nc.vector.memset(ones12, 1.0)

            wvt = []
            for c in range(CT):
                t = w768_pool.tile([128, C], BF16, tag="w768", name=f"wv{c}")
                nc.sync.dma_start(out=t, in_=wv[c, :, :])
                wvt.append(t)

            # ---- per-head q/k projection -----------------------------------
            def qk_project(h, wt):
                """q (rows 0-63) and k (rows 64-127), transposed layout."""
                sts, kts, qds = [], [], []
                for qc in range(QC):
                    pqk = qk_ps.tile([128, 512], F32, tag="qk",
                                     name=f"pqk{h}_{qc}")
                    for c in range(CT):
                        nc.tensor.matmul(
                            pqk, wt[:, c * 128:(c + 1) * 128],
                            xt[c][:, qc * 512:(qc + 1) * 512],
                            start=(c == 0), stop=(c == CT - 1),
                        )
                    st = st_pool.tile([128, 512], BF16, tag="st",
                                      name=f"st{h}_{qc}")
                    nc.vector.tensor_copy(st, pqk)
                    # k rows of EVEN key-chunks also needed at partitions 0-63
                    # (row-tile 0); q rows duplicated at partitions 64-127 for
                    # the odd-chunk matmuls on row-tile 1.
                    kt_t = ktq_pool.tile([128, 512], BF16, tag="ktq",
                                         name=f"kt{h}_{qc}")
                    nc.sync.dma_start(out=kt_t[0:64, :], in_=st[64:128, :])
                    qd = ktq_pool.tile([128, 512], BF16, tag="ktq",
                                       name=f"qd{h}_{qc}")
                    nc.sync.dma_start(out=qd[64:128, :], in_=st[0:64, :])
                    sts.append(st)
                    kts.append(kt_t)
                    qds.append(qd)
                return sts, kts, qds

            # ---- v_aug[tt] builders ---------------------------------------
            # all 8 tiles live for the whole kernel; write the ones columns
            # up-front so the hot loop's DVE queue stays short
            vaug = [vaug_pool.tile([128, VW], BF16, tag="vaug",
                                   name=f"vaug{tt}") for tt in range(KT)]
            for tt in range(KT):
                ones_ap = bass.AP(tensor=vaug[tt].tensor,
                                  offset=vaug[tt].offset + 64,
                                  ap=[vaug[tt].ap[0], [65, H]])
                nc.vector.tensor_copy(ones_ap, ones12)

            def build_vaug(tt):
                pv = sc_ps.tile([128, 1024], F32, tag="sc", name=f"pv{tt}")
                for c in range(CT):
                    xs = xt[c][:, tt * 128:(tt + 1) * 128]
                    nc.tensor.matmul(pv[:, 0:512], xs, wvt[c][:, 0:512],
                                     start=(c == 0), stop=(c == CT - 1))
                    nc.tensor.matmul(pv[:, 512:768], xs, wvt[c][:, 512:768],
                                     start=(c == 0), stop=(c == CT - 1))
                va = vaug[tt]
                dst = bass.AP(tensor=va.tensor, offset=va.offset,
                              ap=[va.ap[0], [65, H], [1, 64]])
                nc.vector.tensor_copy(dst, pv[:, 0:768])

            # ---- output accumulator tiles (c-major, [128, N]) --------------
            outT = [out_pool.tile([128, N], BF16, tag="outT", name=f"outT{i}")
                    for i in range(CT)]

            def emit_sc(slot, qc, kt, sk):
                sts, kts, qds = sk
                cs = slice((kt % 4) * 128, (kt % 4 + 1) * 128)
                if kt % 2 == 0:
                    nc.tensor.matmul(slot, kts[kt // 4][0:64, cs],
                                     sts[qc][0:64, :], start=True, stop=True,
                                     tile_position=(0, 0))
                else:
                    nc.tensor.matmul(slot, sts[kt // 4][64:128, cs],
                                     qds[qc][64:128, :], start=True, stop=True,
                                     tile_position=(64, 0))

            def emit_sc_chunk(h, qc, sk):
                """Scores + exp for one (head, query-chunk): 4 row-tiled
                pairs into [128,1024] supertiles, one Exp each."""
                ets = []
                for g in range(4):
                    ps = sc_ps.tile([128, 1024], F32, tag="sc",
                                    name=f"sc{h}_{qc}_{g}")
                    emit_sc(ps[:, 0:512], qc, 2 * g, sk)
                    emit_sc(ps[:, 512:1024], qc, 2 * g + 1, sk)
                    et = et_pool.tile([128, 1024], BF16, tag="et",
                                      name=f"et{h}_{qc}_{g}")
                    nc.scalar.activation(out=et, in_=ps, func=EXP)
                    ets.append(et)
                return ets

            def emit_av_chunk(h, qc, ets):
                """attn*v accumulation + softmax normalization for a chunk."""
                av = av_ps.tile([128, 512], F32, tag="av", name=f"av{h}_{qc}")
                for kt in range(KT):
                    nc.tensor.matmul(av, vaug[kt][:, h * 65:h * 65 + 128],
                                     ets[kt // 2][:, (kt % 2) * 512:
                                                  (kt % 2 + 1) * 512],
                                     start=(kt == 0), stop=(kt == KT - 1))
                avs = avs_pool.tile([65, 512], F32, tag="avs",
                                    name=f"avs{h}_{qc}")
                nc.vector.tensor_copy(avs, av[0:65, :])
                # row 64 of avs = softmax denominators for this q chunk.
                sm0 = iv_pool.tile([1, 512], F32, tag="sm0",
                                   name=f"sm0{h}_{qc}")
                nc.gpsimd.dma_start(out=sm0, in_=avs[64:65, :])
                iv0 = iv_pool.tile([1, 512], F32, tag="iv0",
                                   name=f"iv0{h}_{qc}")
                nc.vector.reciprocal_approx_fast(out=iv0, in_=sm0)
                bc = bc_pool.tile([64, 512], F32, tag="bc", name=f"bc{h}_{qc}")
                nc.gpsimd.partition_broadcast(bc, iv0)
                ct_i = h // 2
                if h % 2 == 0:
                    nc.vector.tensor_mul(
                        outT[ct_i][0:64, qc * 512:(qc + 1) * 512],
                        avs[0:64, :], bc)
                else:
                    ost = ost_pool.tile([64, 512], BF16, tag="ost",
                                        name=f"ost{h}_{qc}")
                    nc.vector.tensor_mul(ost, avs[0:64, :], bc)
                    nc.gpsimd.dma_start(
                        out=outT[ct_i][64:128, qc * 512:(qc + 1) * 512],
                        in_=ost)

            wptt = None

            def emit_proj(tts):
                for tt in tts:
                    py = sc_ps.tile([128, 1024], F32, tag="sc",
                                    name=f"py{tt}")
                    for c in range(CT):
                        os_ = outT[c][:, tt * 128:(tt + 1) * 128]
                        nc.tensor.matmul(py[:, 0:512], os_, wptt[c][:, 0:512],
                                         start=(c == 0), stop=(c == CT - 1))
                        nc.tensor.matmul(py[:, 512:768], os_,
                                         wptt[c][:, 512:768],
                                         start=(c == 0), stop=(c == CT - 1))
                    ysb = y_pool.tile([128, C], F32, tag="y", name=f"y{tt}")
                    nc.vector.tensor_add(ysb[:, 0:384], py[:, 0:384],
                                         bias_bc[:, 0:384])
                    nc.gpsimd.dma_start(out=y[tt * 128:(tt + 1) * 128, 0:384],
                                        in_=ysb[:, 0:384])
                    nc.vector.tensor_add(ysb[:, 384:768], py[:, 384:768],
                                         bias_bc[:, 384:768])
                    nc.gpsimd.dma_start(out=y[tt * 128:(tt + 1) * 128, 384:768],
                                        in_=ysb[:, 384:768])

            # ---- software-pipelined schedule -------------------------------
            head_order = list(range(H))
            head_order[10], head_order[11] = head_order[11], head_order[10]
            chunks = [(h, qc) for h in head_order for qc in range(QC)]
            pv_slots = {0: (0, 1), 1: (2, 3), 2: (4, 5), 3: (6, 7)}

            sk_by_head = {head_order[0]: qk_project(head_order[0], wts0)}
            ets_store = {}
            av_done = 0

            for s, (h, qc) in enumerate(chunks):
                ets_store[s] = emit_sc_chunk(h, qc, sk_by_head[h])
                if s in pv_slots:
                    for tt in pv_slots[s]:
                        build_vaug(tt)
                # av chunks before qk: their DVE drain chain must precede the
                # qk casts in the FIFO (it releases the av PSUM bank)
                target = 0 if s < 3 else (s - 2 if s < 8 else s)
                while av_done < target:
                    ch, cq = chunks[av_done]
                    emit_av_chunk(ch, cq, ets_store.pop(av_done))
                    av_done += 1
                    if av_done == len(chunks) - 1:
                        emit_proj(range(0, 4))
                if qc == 0 and 2 * (s // 2) + 2 < len(chunks):
                    hn = chunks[s + 2][0]
                    sk_by_head[hn] = qk_project(hn, load_wqk(hn))
                if (h, qc) in ((4, 1), (5, 1)):
                    # prefetch output-projection weights in two half-bursts
                    # on the gpsimd queue (keeps Sync free for kt/qd shifts)
                    if wptt is None:
                        wptt = []
                    for c in range(len(wptt), len(wptt) + 3):
                        t = w768_pool.tile([128, C], BF16, tag="w768",
                                           name=f"wpt{c}")
                        nc.gpsimd.dma_start(out=t, in_=wpt[c, :, :])
                        wptt.append(t)

            while av_done < len(chunks):
                ch, cq = chunks[av_done]
                emit_av_chunk(ch, cq, ets_store.pop(av_done))
                av_done += 1
                if av_done == len(chunks) - 1:
                    emit_proj(range(0, 4))
            emit_proj(range(4, KT))

    nc.finalize()
    return nc


def _get_nc():
    global _NC_CACHE
    if _NC_CACHE is None:
        _NC_CACHE = _build()
    return _NC_CACHE


def _host_prep(x, Wqkv, Wproj, bproj, Aq, Bq, Av, Bv):
    """Fold LoRA + score scale into the weights; lay out and cast to bf16."""
    W = Wqkv.astype(np.float64)
    Wq = W[0:C].reshape(H, HD, C)
    Wk = W[C:2 * C].reshape(H, HD, C)
    Wv_ = W[2 * C:3 * C].reshape(H, HD, C)
    ABq = Aq.astype(np.float64) @ Bq.astype(np.float64)   # [HD, HD]
    ABv = Av.astype(np.float64) @ Bv.astype(np.float64)
    Wq = Wq + np.einsum('ed,hec->hdc', ABq, Wq)           # (I+AB).T @ Wq per head
    Wv_ = Wv_ + np.einsum('ed,hec->hdc', ABv, Wv_)
    Wq = Wq * SCALE                                       # fold softmax scale

    # wqk[h] = [K=c-rows(128), 6 c-tiles of (q_h cols(64) ++ k_h cols(64))]
    wqk = np.empty((H, 128, C), np.float32)
    for h in range(H):
        for c in range(CT):
            cs = slice(c * 128, (c + 1) * 128)
            wqk[h, :, c * 128:c * 128 + 64] = Wq[h][:, cs].T
            wqk[h, :, c * 128 + 64:(c + 1) * 128] = Wk[h][:, cs].T

    # wv[c] = [K=c-rows(128), all 768 v output features]
    WvT = Wv_.reshape(C, C).T.astype(np.float32)          # [c_in, v_out]
    wv = np.ascontiguousarray(WvT.reshape(CT, 128, C))

    # wpt[c] = Wproj.T c-tiles: [K=c(128), e(768)]
    WpT = Wproj.astype(np.float32).T                      # [c, e]
    wpt = np.ascontiguousarray(WpT.reshape(CT, 128, C))

    bf = ml_dtypes.bfloat16
    wqk = wqk.astype(bf)
    wv = wv.astype(bf)
    wpt = wpt.astype(bf)
    bias = bproj.astype(np.float32).reshape(1, C)

    per_core = []
    for b in range(B):
        xTb = np.ascontiguousarray(x[b].astype(np.float32).T.astype(bf))
        per_core.append({"xT": xTb, "wqk": wqk, "wv": wv, "wpt": wpt,
                         "bias": bias})
    return per_core


def kernel(x, Wqkv, Wproj, bproj, Aq, Bq, Av, Bv, _trace=False):
    x = np.asarray(x)
    in_maps = _host_prep(np.asarray(x), np.asarray(Wqkv), np.asarray(Wproj),
                         np.asarray(bproj), np.asarray(Aq), np.asarray(Bq),
                         np.asarray(Av), np.asarray(Bv))
    nc = _get_nc()
    res = run_bass_kernel_spmd(nc, in_maps, core_ids=list(range(N_CORES)),
                               trace=_trace)
    out = np.stack([res.results[b]["y"] for b in range(B)], axis=0)
    if _trace:
        kernel._last_result = res
    return out.astype(np.float32)


# revision 16
# speedup vs baseline: 1.1296x; 1.1296x over previous
"""LoRA attention kernel for Trainium2, batch-sharded across 8 NeuronCores.

Strategy (v4):
  - Data parallel: batch B=8 -> one batch element per core.
  - LoRA factors and the 1/sqrt(hd) score scale are folded into Wqkv on the
    host (exact algebra, float64).
  - All matmul operands are bf16 (PSUM accumulation stays fp32): halves SBUF
    traffic and enables FWL fast weight loads.
  - q,k are produced transposed ([head_dim, tokens]) directly from x^T so the
    score matmuls need no on-chip transposes. v is produced in natural layout
    with an extra all-ones column per head, so the attention-value matmul
    accumulates the softmax denominators for free in row 64 of its output.
  - Score matmuls run K=64 as row-tiled pairs: even key-tiles on PE rows
    0-63 (tile_position (0,0)), odd key-tiles on rows 64-127 ((64,0)).
    Adjacent issue makes each (even,odd) pair execute concurrently in
    disjoint array row-groups (~310ns/pair vs 430ns serial).
  - Scores land in [128, 1024] PSUM supertiles (2 banks, 3 rotating bufs);
    one ACTIVATE(Exp) covers 2 key-tiles, amortizing ScalarE's ~352-cycle
    per-instruction overhead. ScalarE is the pacing engine: the whole kernel
    is software-pipelined so exp inputs are always ready ahead of it --
    score groups of chunk s are emitted while attn*v groups run 1-3 chunks
    behind (v-aug construction fills the early-chunk PE slack).
"""
import numpy as np
import ml_dtypes

import concourse.bass as bass
import concourse.bacc as bacc
import concourse.mybir as mybir
import concourse.tile as tile
from concourse.bass_utils import run_bass_kernel_spmd

F32 = mybir.dt.float32
BF16 = mybir.dt.bfloat16
EXP = mybir.ActivationFunctionType.Exp

B, N, C, H, HD = 8, 1024, 768, 12, 64
CT = C // 128           # 6 contraction tiles over C
QC = N // 512           # 2 query chunks of 512
KT = N // 128           # 8 key tiles of 128
SCALE = HD ** -0.5
N_CORES = 8
VW = (H - 1) * 65 + 128  # vaug tile width (65-pitch heads, widened last read)

_NC_CACHE = None


def _build():
    nc = bacc.Bacc(None, target_bir_lowering=False)

    xT = nc.dram_tensor("xT", [C, N], BF16, kind="ExternalInput")
    wqk = nc.dram_tensor("wqk", [H, 128, C], BF16, kind="ExternalInput")
    wv = nc.dram_tensor("wv", [CT, 128, C], BF16, kind="ExternalInput")
    wpt = nc.dram_tensor("wpt", [CT, 128, C], BF16, kind="ExternalInput")
    bias = nc.dram_tensor("bias", [1, C], F32, kind="ExternalInput")
    y = nc.dram_tensor("y", [N, C], F32, kind="ExternalOutput")

    from contextlib import ExitStack
    with tile.TileContext(nc) as tc:
        with ExitStack() as ctx:
            pool = lambda name, bufs, **kw: ctx.enter_context(
                tc.tile_pool(name=name, bufs=bufs, **kw))
            xt_pool = pool("xt", CT)
            wqk_pool = pool("wqkp", 3)
            w768_pool = pool("w768", 2 * CT)      # wv + wpt
            vaug_pool = pool("vaug", KT)
            st_pool = pool("stp", 6)
            ktq_pool = pool("ktq", 8)
            et_pool = pool("etp", 16)
            avs_pool = pool("avsp", 3)
            iv_pool = pool("ivp", 4)
            bc_pool = pool("bcp", 3)
            ost_pool = pool("ostp", 3)
            out_pool = pool("outp", CT)
            y_pool = pool("yp", 3)
            cst_pool = pool("cst", 1)
            sc_ps = pool("sc_ps", 3, space="PSUM")    # [128,1024] supertiles
            av_ps = pool("av_ps", 1, space="PSUM")
            qk_ps = pool("qk_ps", 1, space="PSUM")

            # ---- PE warm-up: bridge the DMA lead-in so the HAM clock gate
            # opens before real work arrives ---------------------------------
            wur = cst_pool.tile([128, 512], BF16, tag="wur")
            nc.vector.memset(wur, 0.0)
            for i in range(8):
                wps = qk_ps.tile([128, 512], F32, tag="qk", name=f"wu{i}")
                nc.tensor.matmul(wps, wur[:, 0:128], wur,
                                 start=True, stop=True)

            # ---- loads -----------------------------------------------------
            def load_wqk(h):
                wt = wqk_pool.tile([128, C], BF16, tag="wqk", name=f"wqk{h}")
                nc.sync.dma_start(out=wt, in_=wqk[h, :, :])
                return wt

            wts0 = load_wqk(0)

            # x tiles via the Activation queue: parallel to the sync-queue
            # weight loads, and ScalarE is idle during the lead-in anyway
            xt = []
            for c in range(CT):
                t = xt_pool.tile([128, N], BF16, tag="xt", name=f"xt{c}")
                nc.scalar.dma_start(out=t, in_=xT[c * 128:(c + 1) * 128, :])
                xt.append(t)

            bias_bc = cst_pool.tile([128, C], F32, tag="biasbc")
            nc.sync.dma_start(out=bias_bc, in_=bias[:, :].to_broadcast([128, C]))
            ones12 = cst_pool.tile([128, H], BF16, tag="ones12")
            nc.vector.memset(ones12, 1.0)

            wvt = []
            for c in range(CT):
                t = w768_pool.tile([128, C], BF16, tag="w768", name=f"wv{c}")
                nc.sync.dma_start(out=t, in_=wv[c, :, :])
                wvt.append(t)

            # ---- per-head q/k projection -----------------------------------
            def qk_project(h, wt):
                """q (rows 0-63) and k (rows 64-127), transposed layout."""
                sts, kts, qds = [], [], []
                for qc in range(QC):
                    pqk = qk_ps.tile([128, 512], F32, tag="qk",
                                     name=f"pqk{h}_{qc}")
                    for c in range(CT):
                        nc.tensor.matmul(
                            pqk, wt[:, c * 128:(c + 1) * 128],
                            xt[c][:, qc * 512:(qc + 1) * 512],
                            start=(c == 0), stop=(c == CT - 1),
                        )
                    st = st_pool.tile([128, 512], BF16, tag="st",
                                      name=f"st{h}_{qc}")
                    nc.vector.tensor_copy(st, pqk)
                    # k rows of EVEN key-chunks also needed at partitions 0-63
                    # (row-tile 0); q rows duplicated at partitions 64-127 for
                    # the odd-chunk matmuls on row-tile 1.
                    kt_t = ktq_pool.tile([128, 512], BF16, tag="ktq",
                                         name=f"kt{h}_{qc}")
                    nc.sync.dma_start(out=kt_t[0:64, :], in_=st[64:128, :])
                    qd = ktq_pool.tile([128, 512], BF16, tag="ktq",
                                       name=f"qd{h}_{qc}")
                    nc.sync.dma_start(out=qd[64:128, :], in_=st[0:64, :])
                    sts.append(st)
                    kts.append(kt_t)
                    qds.append(qd)
                return sts, kts, qds

            # ---- v_aug[tt] builders ---------------------------------------
            # all 8 tiles live for the whole kernel; write the ones columns
            # up-front so the hot loop's DVE queue stays short
            vaug = [vaug_pool.tile([128, VW], BF16, tag="vaug",
                                   name=f"vaug{tt}") for tt in range(KT)]
            for tt in range(KT):
                ones_ap = bass.AP(tensor=vaug[tt].tensor,
                                  offset=vaug[tt].offset + 64,
                                  ap=[vaug[tt].ap[0], [65, H]])
                nc.vector.tensor_copy(ones_ap, ones12)

            def build_vaug(tt):
                pv = sc_ps.tile([128, 1024], F32, tag="sc", name=f"pv{tt}")
                for c in range(CT):
                    xs = xt[c][:, tt * 128:(tt + 1) * 128]
                    nc.tensor.matmul(pv[:, 0:512], xs, wvt[c][:, 0:512],
                                     start=(c == 0), stop=(c == CT - 1))
                    nc.tensor.matmul(pv[:, 512:768], xs, wvt[c][:, 512:768],
                                     start=(c == 0), stop=(c == CT - 1))
                va = vaug[tt]
                dst = bass.AP(tensor=va.tensor, offset=va.offset,
                              ap=[va.ap[0], [65, H], [1, 64]])
                nc.vector.tensor_copy(dst, pv[:, 0:768])

            # ---- output accumulator tiles (c-major, [128, N]) --------------
            outT = [out_pool.tile([128, N], BF16, tag="outT", name=f"outT{i}")
                    for i in range(CT)]

            def emit_sc(slot, qc, kt, sk):
                sts, kts, qds = sk
                cs = slice((kt % 4) * 128, (kt % 4 + 1) * 128)
                if kt % 2 == 0:
                    nc.tensor.matmul(slot, kts[kt // 4][0:64, cs],
                                     sts[qc][0:64, :], start=True, stop=True,
                                     tile_position=(0, 0))
                else:
                    nc.tensor.matmul(slot, sts[kt // 4][64:128, cs],
                                     qds[qc][64:128, :], start=True, stop=True,
                                     tile_position=(64, 0))

            def emit_sc_chunk(h, qc, sk):
                """Scores + exp for one (head, query-chunk): 4 row-tiled
                pairs into [128,1024] supertiles, one Exp each."""
                ets = []
                for g in range(4):
                    ps = sc_ps.tile([128, 1024], F32, tag="sc",
                                    name=f"sc{h}_{qc}_{g}")
                    emit_sc(ps[:, 0:512], qc, 2 * g, sk)
                    emit_sc(ps[:, 512:1024], qc, 2 * g + 1, sk)
                    et = et_pool.tile([128, 1024], BF16, tag="et",
                                      name=f"et{h}_{qc}_{g}")
                    nc.scalar.activation(out=et, in_=ps, func=EXP)
                    ets.append(et)
                return ets

            def emit_av_chunk(h, qc, ets):
                """attn*v accumulation + softmax normalization for a chunk."""
                av = av_ps.tile([128, 512], F32, tag="av", name=f"av{h}_{qc}")
                for kt in range(KT):
                    nc.tensor.matmul(av, vaug[kt][:, h * 65:h * 65 + 128],
                                     ets[kt // 2][:, (kt % 2) * 512:
                                                  (kt % 2 + 1) * 512],
                                     start=(kt == 0), stop=(kt == KT - 1))
                avs = avs_pool.tile([65, 512], F32, tag="avs",
                                    name=f"avs{h}_{qc}")
                nc.vector.tensor_copy(avs, av[0:65, :])
                # row 64 of avs = softmax denominators for this q chunk.
                sm0 = iv_pool.tile([1, 512], F32, tag="sm0",
                                   name=f"sm0{h}_{qc}")
                nc.sync.dma_start(out=sm0, in_=avs[64:65, :])
                iv0 = iv_pool.tile([1, 512], F32, tag="iv0",
                                   name=f"iv0{h}_{qc}")
                nc.vector.reciprocal_approx_fast(out=iv0, in_=sm0)
                bc = bc_pool.tile([64, 512], F32, tag="bc", name=f"bc{h}_{qc}")
                nc.gpsimd.partition_broadcast(bc, iv0)
                ct_i = h // 2
                if h % 2 == 0:
                    nc.vector.tensor_mul(
                        outT[ct_i][0:64, qc * 512:(qc + 1) * 512],
                        avs[0:64, :], bc)
                else:
                    ost = ost_pool.tile([64, 512], BF16, tag="ost",
                                        name=f"ost{h}_{qc}")
                    nc.vector.tensor_mul(ost, avs[0:64, :], bc)
                    nc.sync.dma_start(
                        out=outT[ct_i][64:128, qc * 512:(qc + 1) * 512],
                        in_=ost)

            wptt = None

            def emit_proj(tts):
                for tt in tts:
                    py = sc_ps.tile([128, 1024], F32, tag="sc",
                                    name=f"py{tt}")
                    for c in range(CT):
                        os_ = outT[c][:, tt * 128:(tt + 1) * 128]
                        nc.tensor.matmul(py[:, 0:512], os_, wptt[c][:, 0:512],
                                         start=(c == 0), stop=(c == CT - 1))
                        nc.tensor.matmul(py[:, 512:768], os_,
                                         wptt[c][:, 512:768],
                                         start=(c == 0), stop=(c == CT - 1))
                    ysb = y_pool.tile([128, C], F32, tag="y", name=f"y{tt}")
                    nc.vector.tensor_add(ysb[:, 0:384], py[:, 0:384],
                                         bias_bc[:, 0:384])
                    nc.sync.dma_start(out=y[tt * 128:(tt + 1) * 128, 0:384],
                                        in_=ysb[:, 0:384])
                    nc.vector.tensor_add(ysb[:, 384:768], py[:, 384:768],
                                         bias_bc[:, 384:768])
                    nc.sync.dma_start(out=y[tt * 128:(tt + 1) * 128, 384:768],
                                        in_=ysb[:, 384:768])

            # ---- software-pipelined schedule -------------------------------
            head_order = list(range(H))
            head_order[10], head_order[11] = head_order[11], head_order[10]
            chunks = [(h, qc) for h in head_order for qc in range(QC)]
            pv_slots = {0: (0, 1), 1: (2, 3), 2: (4, 5), 3: (6, 7)}

            sk_by_head = {head_order[0]: qk_project(head_order[0], wts0)}
            ets_store = {}
            av_done = 0

            for s, (h, qc) in enumerate(chunks):
                ets_store[s] = emit_sc_chunk(h, qc, sk_by_head[h])
                if s in pv_slots:
                    for tt in pv_slots[s]:
                        build_vaug(tt)
                # av chunks before qk: their DVE drain chain must precede the
                # qk casts in the FIFO (it releases the av PSUM bank)
                target = 0 if s < 3 else (s - 2 if s < 8 else s)
                while av_done < target:
                    ch, cq = chunks[av_done]
                    emit_av_chunk(ch, cq, ets_store.pop(av_done))
                    av_done += 1
                    if av_done == len(chunks) - 1:
                        emit_proj(range(0, 4))
                if qc == 0 and 2 * (s // 2) + 2 < len(chunks):
                    hn = chunks[s + 2][0]
                    sk_by_head[hn] = qk_project(hn, load_wqk(hn))
                if (h, qc) in ((4, 1), (5, 1)):
                    # prefetch output-projection weights in two half-bursts
                    # on the sync queue
                    if wptt is None:
                        wptt = []
                    for c in range(len(wptt), len(wptt) + 3):
                        t = w768_pool.tile([128, C], BF16, tag="w768",
                                           name=f"wpt{c}")
                        nc.sync.dma_start(out=t, in_=wpt[c, :, :])
                        wptt.append(t)

            while av_done < len(chunks):
                ch, cq = chunks[av_done]
                emit_av_chunk(ch, cq, ets_store.pop(av_done))
                av_done += 1
                if av_done == len(chunks) - 1:
                    emit_proj(range(0, 4))
            emit_proj(range(4, KT))

    nc.finalize()
    return nc


def _get_nc():
    global _NC_CACHE
    if _NC_CACHE is None:
        _NC_CACHE = _build()
    return _NC_CACHE


def _host_prep(x, Wqkv, Wproj, bproj, Aq, Bq, Av, Bv):
    """Fold LoRA + score scale into the weights; lay out and cast to bf16."""
    W = Wqkv.astype(np.float64)
    Wq = W[0:C].reshape(H, HD, C)
    Wk = W[C:2 * C].reshape(H, HD, C)
    Wv_ = W[2 * C:3 * C].reshape(H, HD, C)
    ABq = Aq.astype(np.float64) @ Bq.astype(np.float64)   # [HD, HD]
    ABv = Av.astype(np.float64) @ Bv.astype(np.float64)
    Wq = Wq + np.einsum('ed,hec->hdc', ABq, Wq)           # (I+AB).T @ Wq per head
    Wv_ = Wv_ + np.einsum('ed,hec->hdc', ABv, Wv_)
    Wq = Wq * SCALE                                       # fold softmax scale

    # wqk[h] = [K=c-rows(128), 6 c-tiles of (q_h cols(64) ++ k_h cols(64))]
    wqk = np.empty((H, 128, C), np.float32)
    for h in range(H):
        for c in range(CT):
            cs = slice(c * 128, (c + 1) * 128)
            wqk[h, :, c * 128:c * 128 + 64] = Wq[h][:, cs].T
            wqk[h, :, c * 128 + 64:(c + 1) * 128] = Wk[h][:, cs].T

    # wv[c] = [K=c-rows(128), all 768 v output features]
    WvT = Wv_.reshape(C, C).T.astype(np.float32)          # [c_in, v_out]
    wv = np.ascontiguousarray(WvT.reshape(CT, 128, C))

    # wpt[c] = Wproj.T c-tiles: [K=c(128), e(768)]
    WpT = Wproj.astype(np.float32).T                      # [c, e]
    wpt = np.ascontiguousarray(WpT.reshape(CT, 128, C))

    bf = ml_dtypes.bfloat16
    wqk = wqk.astype(bf)
    wv = wv.astype(bf)
    wpt = wpt.astype(bf)
    bias = bproj.astype(np.float32).reshape(1, C)

    per_core = []
    for b in range(B):
        xTb = np.ascontiguousarray(x[b].astype(np.float32).T.astype(bf))
        per_core.append({"xT": xTb, "wqk": wqk, "wv": wv, "wpt": wpt,
                         "bias": bias})
    return per_core


def kernel(x, Wqkv, Wproj, bproj, Aq, Bq, Av, Bv, _trace=False):
    x = np.asarray(x)
    in_maps = _host_prep(np.asarray(x), np.asarray(Wqkv), np.asarray(Wproj),
                         np.asarray(bproj), np.asarray(Aq), np.asarray(Bq),
                         np.asarray(Av), np.asarray(Bv))
    nc = _get_nc()
    res = run_bass_kernel_spmd(nc, in_maps, core_ids=list(range(N_CORES)),
                               trace=_trace)
    out = np.stack([res.results[b]["y"] for b in range(B)], axis=0)
    if _trace:
        kernel._last_result = res
    return out.astype(np.float32)


# revision 17
# speedup vs baseline: 1.1391x; 1.0085x over previous
"""LoRA attention kernel for Trainium2, batch-sharded across 8 NeuronCores.

Strategy (v4):
  - Data parallel: batch B=8 -> one batch element per core.
  - LoRA factors and the 1/sqrt(hd) score scale are folded into Wqkv on the
    host (exact algebra, float64).
  - All matmul operands are bf16 (PSUM accumulation stays fp32): halves SBUF
    traffic and enables FWL fast weight loads.
  - q,k are produced transposed ([head_dim, tokens]) directly from x^T so the
    score matmuls need no on-chip transposes. v is produced in natural layout
    with an extra all-ones column per head, so the attention-value matmul
    accumulates the softmax denominators for free in row 64 of its output.
  - Score matmuls run K=64 as row-tiled pairs: even key-tiles on PE rows
    0-63 (tile_position (0,0)), odd key-tiles on rows 64-127 ((64,0)).
    Adjacent issue makes each (even,odd) pair execute concurrently in
    disjoint array row-groups (~310ns/pair vs 430ns serial).
  - Scores land in [128, 1024] PSUM supertiles (2 banks, 3 rotating bufs);
    one ACTIVATE(Exp) covers 2 key-tiles, amortizing ScalarE's ~352-cycle
    per-instruction overhead. ScalarE is the pacing engine: the whole kernel
    is software-pipelined so exp inputs are always ready ahead of it --
    score groups of chunk s are emitted while attn*v groups run 1-3 chunks
    behind (v-aug construction fills the early-chunk PE slack).
"""
import numpy as np
import ml_dtypes

import concourse.bass as bass
import concourse.bacc as bacc
import concourse.mybir as mybir
import concourse.tile as tile
from concourse.bass_utils import run_bass_kernel_spmd

F32 = mybir.dt.float32
BF16 = mybir.dt.bfloat16
EXP = mybir.ActivationFunctionType.Exp

B, N, C, H, HD = 8, 1024, 768, 12, 64
CT = C // 128           # 6 contraction tiles over C
QC = N // 512           # 2 query chunks of 512
KT = N // 128           # 8 key tiles of 128
SCALE = HD ** -0.5
N_CORES = 8
VW = (H - 1) * 65 + 128  # vaug tile width (65-pitch heads, widened last read)

_NC_CACHE = None


def _build():
    nc = bacc.Bacc(None, target_bir_lowering=False)

    xT = nc.dram_tensor("xT", [C, N], BF16, kind="ExternalInput")
    wqk = nc.dram_tensor("wqk", [H, 128, C], BF16, kind="ExternalInput")
    wv = nc.dram_tensor("wv", [CT, 128, C], BF16, kind="ExternalInput")
    wpt = nc.dram_tensor("wpt", [CT, 128, C], BF16, kind="ExternalInput")
    bias = nc.dram_tensor("bias", [1, C], F32, kind="ExternalInput")
    y = nc.dram_tensor("y", [N, C], F32, kind="ExternalOutput")

    from contextlib import ExitStack
    with tile.TileContext(nc) as tc:
        with ExitStack() as ctx:
            pool = lambda name, bufs, **kw: ctx.enter_context(
                tc.tile_pool(name=name, bufs=bufs, **kw))
            xt_pool = pool("xt", CT)
            wqk_pool = pool("wqkp", 3)
            w768_pool = pool("w768", 2 * CT)      # wv + wpt
            vaug_pool = pool("vaug", KT)
            st_pool = pool("stp", 6)
            ktq_pool = pool("ktq", 8)
            et_pool = pool("etp", 16)
            avs_pool = pool("avsp", 3)
            iv_pool = pool("ivp", 4)
            bc_pool = pool("bcp", 3)
            ost_pool = pool("ostp", 3)
            out_pool = pool("outp", CT)
            y_pool = pool("yp", 3)
            cst_pool = pool("cst", 1)
            sc_ps = pool("sc_ps", 3, space="PSUM")    # [128,1024] supertiles
            av_ps = pool("av_ps", 1, space="PSUM")
            qk_ps = pool("qk_ps", 1, space="PSUM")

            # ---- PE warm-up: bridge the DMA lead-in so the HAM clock gate
            # opens before real work arrives ---------------------------------
            wur = cst_pool.tile([128, 512], BF16, tag="wur")
            nc.vector.memset(wur, 0.0)
            for i in range(10):
                wps = qk_ps.tile([128, 512], F32, tag="qk", name=f"wu{i}")
                nc.tensor.matmul(wps, wur[:, 0:128], wur,
                                 start=True, stop=True)

            # ---- loads -----------------------------------------------------
            def load_wqk(h):
                wt = wqk_pool.tile([128, C], BF16, tag="wqk", name=f"wqk{h}")
                nc.sync.dma_start(out=wt, in_=wqk[h, :, :])
                return wt

            wts0 = load_wqk(0)

            # x tiles via the Activation queue: parallel to the sync-queue
            # weight loads, and ScalarE is idle during the lead-in anyway
            xt = []
            for c in range(CT):
                t = xt_pool.tile([128, N], BF16, tag="xt", name=f"xt{c}")
                nc.scalar.dma_start(out=t, in_=xT[c * 128:(c + 1) * 128, :])
                xt.append(t)

            bias_bc = cst_pool.tile([128, C], F32, tag="biasbc")
            nc.sync.dma_start(out=bias_bc, in_=bias[:, :].to_broadcast([128, C]))
            ones12 = cst_pool.tile([128, H], BF16, tag="ones12")
            nc.vector.memset(ones12, 1.0)

            wvt = []
            for c in range(CT):
                t = w768_pool.tile([128, C], BF16, tag="w768", name=f"wv{c}")
                nc.sync.dma_start(out=t, in_=wv[c, :, :])
                wvt.append(t)

            # ---- per-head q/k projection -----------------------------------
            def qk_project(h, wt):
                """q (rows 0-63) and k (rows 64-127), transposed layout."""
                sts, kts, qds = [], [], []
                for qc in range(QC):
                    pqk = qk_ps.tile([128, 512], F32, tag="qk",
                                     name=f"pqk{h}_{qc}")
                    for c in range(CT):
                        nc.tensor.matmul(
                            pqk, wt[:, c * 128:(c + 1) * 128],
                            xt[c][:, qc * 512:(qc + 1) * 512],
                            start=(c == 0), stop=(c == CT - 1),
                        )
                    st = st_pool.tile([128, 512], BF16, tag="st",
                                      name=f"st{h}_{qc}")
                    nc.vector.tensor_copy(st, pqk)
                    # k rows of EVEN key-chunks also needed at partitions 0-63
                    # (row-tile 0); q rows duplicated at partitions 64-127 for
                    # the odd-chunk matmuls on row-tile 1.
                    kt_t = ktq_pool.tile([128, 512], BF16, tag="ktq",
                                         name=f"kt{h}_{qc}")
                    nc.sync.dma_start(out=kt_t[0:64, :], in_=st[64:128, :])
                    qd = ktq_pool.tile([128, 512], BF16, tag="ktq",
                                       name=f"qd{h}_{qc}")
                    nc.sync.dma_start(out=qd[64:128, :], in_=st[0:64, :])
                    sts.append(st)
                    kts.append(kt_t)
                    qds.append(qd)
                return sts, kts, qds

            # ---- v_aug[tt] builders ---------------------------------------
            # all 8 tiles live for the whole kernel; write the ones columns
            # up-front so the hot loop's DVE queue stays short
            vaug = [vaug_pool.tile([128, VW], BF16, tag="vaug",
                                   name=f"vaug{tt}") for tt in range(KT)]
            for tt in range(KT):
                ones_ap = bass.AP(tensor=vaug[tt].tensor,
                                  offset=vaug[tt].offset + 64,
                                  ap=[vaug[tt].ap[0], [65, H]])
                nc.vector.tensor_copy(ones_ap, ones12)

            def build_vaug(tt):
                pv = sc_ps.tile([128, 1024], F32, tag="sc", name=f"pv{tt}")
                for c in range(CT):
                    xs = xt[c][:, tt * 128:(tt + 1) * 128]
                    nc.tensor.matmul(pv[:, 0:512], xs, wvt[c][:, 0:512],
                                     start=(c == 0), stop=(c == CT - 1))
                    nc.tensor.matmul(pv[:, 512:768], xs, wvt[c][:, 512:768],
                                     start=(c == 0), stop=(c == CT - 1))
                va = vaug[tt]
                dst = bass.AP(tensor=va.tensor, offset=va.offset,
                              ap=[va.ap[0], [65, H], [1, 64]])
                nc.vector.tensor_copy(dst, pv[:, 0:768])

            # ---- output accumulator tiles (c-major, [128, N]) --------------
            outT = [out_pool.tile([128, N], BF16, tag="outT", name=f"outT{i}")
                    for i in range(CT)]

            def emit_sc(slot, qc, kt, sk):
                sts, kts, qds = sk
                cs = slice((kt % 4) * 128, (kt % 4 + 1) * 128)
                if kt % 2 == 0:
                    nc.tensor.matmul(slot, kts[kt // 4][0:64, cs],
                                     sts[qc][0:64, :], start=True, stop=True,
                                     tile_position=(0, 0))
                else:
                    nc.tensor.matmul(slot, sts[kt // 4][64:128, cs],
                                     qds[qc][64:128, :], start=True, stop=True,
                                     tile_position=(64, 0))

            def emit_sc_chunk(h, qc, sk):
                """Scores + exp for one (head, query-chunk): 4 row-tiled
                pairs into [128,1024] supertiles, one Exp each."""
                ets = []
                for g in range(4):
                    ps = sc_ps.tile([128, 1024], F32, tag="sc",
                                    name=f"sc{h}_{qc}_{g}")
                    emit_sc(ps[:, 0:512], qc, 2 * g, sk)
                    emit_sc(ps[:, 512:1024], qc, 2 * g + 1, sk)
                    et = et_pool.tile([128, 1024], BF16, tag="et",
                                      name=f"et{h}_{qc}_{g}")
                    nc.scalar.activation(out=et, in_=ps, func=EXP)
                    ets.append(et)
                return ets

            def emit_av_chunk(h, qc, ets):
                """attn*v accumulation + softmax normalization for a chunk."""
                av = av_ps.tile([128, 512], F32, tag="av", name=f"av{h}_{qc}")
                for kt in range(KT):
                    nc.tensor.matmul(av, vaug[kt][:, h * 65:h * 65 + 128],
                                     ets[kt // 2][:, (kt % 2) * 512:
                                                  (kt % 2 + 1) * 512],
                                     start=(kt == 0), stop=(kt == KT - 1))
                avs = avs_pool.tile([65, 512], F32, tag="avs",
                                    name=f"avs{h}_{qc}")
                nc.vector.tensor_copy(avs, av[0:65, :])
                # row 64 of avs = softmax denominators for this q chunk.
                sm0 = iv_pool.tile([1, 512], F32, tag="sm0",
                                   name=f"sm0{h}_{qc}")
                nc.sync.dma_start(out=sm0, in_=avs[64:65, :])
                iv0 = iv_pool.tile([1, 512], F32, tag="iv0",
                                   name=f"iv0{h}_{qc}")
                nc.vector.reciprocal_approx_fast(out=iv0, in_=sm0)
                bc = bc_pool.tile([64, 512], F32, tag="bc", name=f"bc{h}_{qc}")
                nc.gpsimd.partition_broadcast(bc, iv0)
                ct_i = h // 2
                if h % 2 == 0:
                    nc.vector.tensor_mul(
                        outT[ct_i][0:64, qc * 512:(qc + 1) * 512],
                        avs[0:64, :], bc)
                else:
                    ost = ost_pool.tile([64, 512], BF16, tag="ost",
                                        name=f"ost{h}_{qc}")
                    nc.vector.tensor_mul(ost, avs[0:64, :], bc)
                    nc.sync.dma_start(
                        out=outT[ct_i][64:128, qc * 512:(qc + 1) * 512],
                        in_=ost)

            wptt = None

            def emit_proj(tts):
                for tt in tts:
                    py = sc_ps.tile([128, 1024], F32, tag="sc",
                                    name=f"py{tt}")
                    for c in range(CT):
                        os_ = outT[c][:, tt * 128:(tt + 1) * 128]
                        nc.tensor.matmul(py[:, 0:512], os_, wptt[c][:, 0:512],
                                         start=(c == 0), stop=(c == CT - 1))
                        nc.tensor.matmul(py[:, 512:768], os_,
                                         wptt[c][:, 512:768],
                                         start=(c == 0), stop=(c == CT - 1))
                    ysb = y_pool.tile([128, C], F32, tag="y", name=f"y{tt}")
                    nc.vector.tensor_add(ysb, py[:, 0:768], bias_bc)
                    nc.sync.dma_start(out=y[tt * 128:(tt + 1) * 128, :],
                                      in_=ysb)

            # ---- software-pipelined schedule -------------------------------
            head_order = list(range(H))
            head_order[10], head_order[11] = head_order[11], head_order[10]
            chunks = [(h, qc) for h in head_order for qc in range(QC)]
            pv_slots = {0: (0, 1), 1: (2, 3), 2: (4, 5), 3: (6, 7)}

            sk_by_head = {head_order[0]: qk_project(head_order[0], wts0)}
            ets_store = {}
            av_done = 0

            for s, (h, qc) in enumerate(chunks):
                ets_store[s] = emit_sc_chunk(h, qc, sk_by_head[h])
                if s in pv_slots:
                    for tt in pv_slots[s]:
                        build_vaug(tt)
                # av chunks before qk: their DVE drain chain must precede the
                # qk casts in the FIFO (it releases the av PSUM bank)
                target = 0 if s < 3 else (s - 2 if s < 8 else s)
                while av_done < target:
                    ch, cq = chunks[av_done]
                    emit_av_chunk(ch, cq, ets_store.pop(av_done))
                    av_done += 1
                    if av_done == len(chunks) - 1:
                        emit_proj(range(0, 4))
                if qc == 0 and 2 * (s // 2) + 2 < len(chunks):
                    hn = chunks[s + 2][0]
                    sk_by_head[hn] = qk_project(hn, load_wqk(hn))
                if (h, qc) in ((4, 1), (5, 1)):
                    # prefetch output-projection weights in two half-bursts
                    # on the sync queue
                    if wptt is None:
                        wptt = []
                    for c in range(len(wptt), len(wptt) + 3):
                        t = w768_pool.tile([128, C], BF16, tag="w768",
                                           name=f"wpt{c}")
                        nc.sync.dma_start(out=t, in_=wpt[c, :, :])
                        wptt.append(t)

            while av_done < len(chunks):
                ch, cq = chunks[av_done]
                emit_av_chunk(ch, cq, ets_store.pop(av_done))
                av_done += 1
                if av_done == len(chunks) - 1:
                    emit_proj(range(0, 4))
            emit_proj(range(4, KT))

    nc.finalize()
    return nc


def _get_nc():
    global _NC_CACHE
    if _NC_CACHE is None:
        _NC_CACHE = _build()
    return _NC_CACHE


def _host_prep(x, Wqkv, Wproj, bproj, Aq, Bq, Av, Bv):
    """Fold LoRA + score scale into the weights; lay out and cast to bf16."""
    W = Wqkv.astype(np.float64)
    Wq = W[0:C].reshape(H, HD, C)
    Wk = W[C:2 * C].reshape(H, HD, C)
    Wv_ = W[2 * C:3 * C].reshape(H, HD, C)
    ABq = Aq.astype(np.float64) @ Bq.astype(np.float64)   # [HD, HD]
    ABv = Av.astype(np.float64) @ Bv.astype(np.float64)
    Wq = Wq + np.einsum('ed,hec->hdc', ABq, Wq)           # (I+AB).T @ Wq per head
    Wv_ = Wv_ + np.einsum('ed,hec->hdc', ABv, Wv_)
    Wq = Wq * SCALE                                       # fold softmax scale

    # wqk[h] = [K=c-rows(128), 6 c-tiles of (q_h cols(64) ++ k_h cols(64))]
    wqk = np.empty((H, 128, C), np.float32)
    for h in range(H):
        for c in range(CT):
            cs = slice(c * 128, (c + 1) * 128)
            wqk[h, :, c * 128:c * 128 + 64] = Wq[h][:, cs].T
            wqk[h, :, c * 128 + 64:(c + 1) * 128] = Wk[h][:, cs].T

    # wv[c] = [K=c-rows(128), all 768 v output features]
    WvT = Wv_.reshape(C, C).T.astype(np.float32)          # [c_in, v_out]
    wv = np.ascontiguousarray(WvT.reshape(CT, 128, C))

    # wpt[c] = Wproj.T c-tiles: [K=c(128), e(768)]
    WpT = Wproj.astype(np.float32).T                      # [c, e]
    wpt = np.ascontiguousarray(WpT.reshape(CT, 128, C))

    bf = ml_dtypes.bfloat16
    wqk = wqk.astype(bf)
    wv = wv.astype(bf)
    wpt = wpt.astype(bf)
    bias = bproj.astype(np.float32).reshape(1, C)

    per_core = []
    for b in range(B):
        xTb = np.ascontiguousarray(x[b].astype(np.float32).T.astype(bf))
        per_core.append({"xT": xTb, "wqk": wqk, "wv": wv, "wpt": wpt,
                         "bias": bias})
    return per_core


def kernel(x, Wqkv, Wproj, bproj, Aq, Bq, Av, Bv, _trace=False):
    x = np.asarray(x)
    in_maps = _host_prep(np.asarray(x), np.asarray(Wqkv), np.asarray(Wproj),
                         np.asarray(bproj), np.asarray(Aq), np.asarray(Bq),
                         np.asarray(Av), np.asarray(Bv))
    nc = _get_nc()
    res = run_bass_kernel_spmd(nc, in_maps, core_ids=list(range(N_CORES)),
                               trace=_trace)
    out = np.stack([res.results[b]["y"] for b in range(B)], axis=0)
    if _trace:
        kernel._last_result = res
    return out.astype(np.float32)


# revision 18
# speedup vs baseline: 1.1601x; 1.0185x over previous
"""LoRA attention kernel for Trainium2, batch-sharded across 8 NeuronCores.

Strategy (v4):
  - Data parallel: batch B=8 -> one batch element per core.
  - LoRA factors and the 1/sqrt(hd) score scale are folded into Wqkv on the
    host (exact algebra, float64).
  - All matmul operands are bf16 (PSUM accumulation stays fp32): halves SBUF
    traffic and enables FWL fast weight loads.
  - q,k are produced transposed ([head_dim, tokens]) directly from x^T so the
    score matmuls need no on-chip transposes. v is produced in natural layout
    with an extra all-ones column per head, so the attention-value matmul
    accumulates the softmax denominators for free in row 64 of its output.
  - Score matmuls run K=64 as row-tiled pairs: even key-tiles on PE rows
    0-63 (tile_position (0,0)), odd key-tiles on rows 64-127 ((64,0)).
    Adjacent issue makes each (even,odd) pair execute concurrently in
    disjoint array row-groups (~310ns/pair vs 430ns serial).
  - Scores land in [128, 1024] PSUM supertiles (2 banks, 3 rotating bufs);
    one ACTIVATE(Exp) covers 2 key-tiles, amortizing ScalarE's ~352-cycle
    per-instruction overhead. ScalarE is the pacing engine: the whole kernel
    is software-pipelined so exp inputs are always ready ahead of it --
    score groups of chunk s are emitted while attn*v groups run 1-3 chunks
    behind (v-aug construction fills the early-chunk PE slack).
"""
import numpy as np
import ml_dtypes

import concourse.bass as bass
import concourse.bacc as bacc
import concourse.mybir as mybir
import concourse.tile as tile
from concourse.bass_utils import run_bass_kernel_spmd

F32 = mybir.dt.float32
BF16 = mybir.dt.bfloat16
EXP = mybir.ActivationFunctionType.Exp

B, N, C, H, HD = 8, 1024, 768, 12, 64
CT = C // 128           # 6 contraction tiles over C
QC = N // 512           # 2 query chunks of 512
KT = N // 128           # 8 key tiles of 128
SCALE = HD ** -0.5
N_CORES = 8
VW = (H - 1) * 65 + 128  # vaug tile width (65-pitch heads, widened last read)

_NC_CACHE = None


def _build():
    nc = bacc.Bacc(None, target_bir_lowering=False)

    xT = nc.dram_tensor("xT", [C, N], BF16, kind="ExternalInput")
    wqk = nc.dram_tensor("wqk", [H, 128, C], BF16, kind="ExternalInput")
    wv = nc.dram_tensor("wv", [CT, 128, C], BF16, kind="ExternalInput")
    wpt = nc.dram_tensor("wpt", [CT, 128, C], BF16, kind="ExternalInput")
    bias = nc.dram_tensor("bias", [1, C], F32, kind="ExternalInput")
    y = nc.dram_tensor("y", [N, C], F32, kind="ExternalOutput")

    from contextlib import ExitStack
    with tile.TileContext(nc) as tc:
        with ExitStack() as ctx:
            pool = lambda name, bufs, **kw: ctx.enter_context(
                tc.tile_pool(name=name, bufs=bufs, **kw))
            xt_pool = pool("xt", CT)
            wqk_pool = pool("wqkp", 3)
            w768_pool = pool("w768", 2 * CT)      # wv + wpt
            vaug_pool = pool("vaug", KT)
            st_pool = pool("stp", 6)
            ktq_pool = pool("ktq", 8)
            et_pool = pool("etp", 16)
            avs_pool = pool("avsp", 3)
            iv_pool = pool("ivp", 4)
            bc_pool = pool("bcp", 3)
            ost_pool = pool("ostp", 3)
            out_pool = pool("outp", CT)
            y_pool = pool("yp", 3)
            cst_pool = pool("cst", 1)
            sc_ps = pool("sc_ps", 3, space="PSUM")    # [128,1024] supertiles
            av_ps = pool("av_ps", 1, space="PSUM")
            qk_ps = pool("qk_ps", 1, space="PSUM")

            # ---- PE warm-up: bridge the DMA lead-in so the HAM clock gate
            # opens before real work arrives ---------------------------------
            wur = cst_pool.tile([128, 512], BF16, tag="wur")
            nc.vector.memset(wur, 0.0)
            for i in range(10):
                wps = qk_ps.tile([128, 512], F32, tag="qk", name=f"wu{i}")
                nc.tensor.matmul(wps, wur[:, 0:128], wur,
                                 start=True, stop=True)

            # ---- loads -----------------------------------------------------
            def load_wqk(h):
                wt = wqk_pool.tile([128, C], BF16, tag="wqk", name=f"wqk{h}")
                nc.sync.dma_start(out=wt, in_=wqk[h, :, :])
                return wt

            wts0 = load_wqk(0)

            xt = []
            for c in range(CT):
                t = xt_pool.tile([128, N], BF16, tag="xt", name=f"xt{c}")
                nc.sync.dma_start(out=t, in_=xT[c * 128:(c + 1) * 128, :])
                xt.append(t)

            bias_bc = cst_pool.tile([128, C], F32, tag="biasbc")
            nc.sync.dma_start(out=bias_bc, in_=bias[:, :].to_broadcast([128, C]))
            ones12 = cst_pool.tile([128, H], BF16, tag="ones12")
            nc.vector.memset(ones12, 1.0)

            wvt = []
            for c in range(CT):
                t = w768_pool.tile([128, C], BF16, tag="w768", name=f"wv{c}")
                nc.sync.dma_start(out=t, in_=wv[c, :, :])
                wvt.append(t)

            # ---- per-head q/k projection -----------------------------------
            def qk_project(h, wt):
                """q (rows 0-63) and k (rows 64-127), transposed layout."""
                sts, kts, qds = [], [], []
                for qc in range(QC):
                    pqk = qk_ps.tile([128, 512], F32, tag="qk",
                                     name=f"pqk{h}_{qc}")
                    for c in range(CT):
                        nc.tensor.matmul(
                            pqk, wt[:, c * 128:(c + 1) * 128],
                            xt[c][:, qc * 512:(qc + 1) * 512],
                            start=(c == 0), stop=(c == CT - 1),
                        )
                    st = st_pool.tile([128, 512], BF16, tag="st",
                                      name=f"st{h}_{qc}")
                    nc.vector.tensor_copy(st, pqk)
                    # k rows of EVEN key-chunks also needed at partitions 0-63
                    # (row-tile 0); q rows duplicated at partitions 64-127 for
                    # the odd-chunk matmuls on row-tile 1.
                    kt_t = ktq_pool.tile([128, 512], BF16, tag="ktq",
                                         name=f"kt{h}_{qc}")
                    nc.sync.dma_start(out=kt_t[0:64, :], in_=st[64:128, :])
                    qd = ktq_pool.tile([128, 512], BF16, tag="ktq",
                                       name=f"qd{h}_{qc}")
                    nc.sync.dma_start(out=qd[64:128, :], in_=st[0:64, :])
                    sts.append(st)
                    kts.append(kt_t)
                    qds.append(qd)
                return sts, kts, qds

            # ---- v_aug[tt] builders ---------------------------------------
            # all 8 tiles live for the whole kernel; write the ones columns
            # up-front so the hot loop's DVE queue stays short
            vaug = [vaug_pool.tile([128, VW], BF16, tag="vaug",
                                   name=f"vaug{tt}") for tt in range(KT)]
            for tt in range(KT):
                ones_ap = bass.AP(tensor=vaug[tt].tensor,
                                  offset=vaug[tt].offset + 64,
                                  ap=[vaug[tt].ap[0], [65, H]])
                nc.vector.tensor_copy(ones_ap, ones12)

            def build_vaug(tt):
                pv = sc_ps.tile([128, 1024], F32, tag="sc", name=f"pv{tt}")
                for c in range(CT):
                    xs = xt[c][:, tt * 128:(tt + 1) * 128]
                    nc.tensor.matmul(pv[:, 0:512], xs, wvt[c][:, 0:512],
                                     start=(c == 0), stop=(c == CT - 1))
                    nc.tensor.matmul(pv[:, 512:768], xs, wvt[c][:, 512:768],
                                     start=(c == 0), stop=(c == CT - 1))
                va = vaug[tt]
                dst = bass.AP(tensor=va.tensor, offset=va.offset,
                              ap=[va.ap[0], [65, H], [1, 64]])
                nc.vector.tensor_copy(dst, pv[:, 0:768])

            # ---- output accumulator tiles (c-major, [128, N]) --------------
            outT = [out_pool.tile([128, N], BF16, tag="outT", name=f"outT{i}")
                    for i in range(CT)]

            def emit_sc(slot, qc, kt, sk):
                sts, kts, qds = sk
                cs = slice((kt % 4) * 128, (kt % 4 + 1) * 128)
                if kt % 2 == 0:
                    nc.tensor.matmul(slot, kts[kt // 4][0:64, cs],
                                     sts[qc][0:64, :], start=True, stop=True,
                                     tile_position=(0, 0))
                else:
                    nc.tensor.matmul(slot, sts[kt // 4][64:128, cs],
                                     qds[qc][64:128, :], start=True, stop=True,
                                     tile_position=(64, 0))

            def emit_sc_chunk(h, qc, sk):
                """Scores + exp for one (head, query-chunk): 4 row-tiled
                pairs into [128,1024] supertiles, one Exp each."""
                ets = []
                for g in range(4):
                    ps = sc_ps.tile([128, 1024], F32, tag="sc",
                                    name=f"sc{h}_{qc}_{g}")
                    emit_sc(ps[:, 0:512], qc, 2 * g, sk)
                    emit_sc(ps[:, 512:1024], qc, 2 * g + 1, sk)
                    et = et_pool.tile([128, 1024], BF16, tag="et",
                                      name=f"et{h}_{qc}_{g}")
                    nc.scalar.activation(out=et, in_=ps, func=EXP)
                    ets.append(et)
                return ets

            def emit_av_chunk(h, qc, ets):
                """attn*v accumulation + softmax normalization for a chunk."""
                av = av_ps.tile([128, 512], F32, tag="av", name=f"av{h}_{qc}")
                for kt in range(KT):
                    nc.tensor.matmul(av, vaug[kt][:, h * 65:h * 65 + 128],
                                     ets[kt // 2][:, (kt % 2) * 512:
                                                  (kt % 2 + 1) * 512],
                                     start=(kt == 0), stop=(kt == KT - 1))
                avs = avs_pool.tile([65, 512], F32, tag="avs",
                                    name=f"avs{h}_{qc}")
                nc.vector.tensor_copy(avs, av[0:65, :])
                # row 64 of avs = softmax denominators for this q chunk.
                sm0 = iv_pool.tile([1, 512], F32, tag="sm0",
                                   name=f"sm0{h}_{qc}")
                nc.sync.dma_start(out=sm0, in_=avs[64:65, :])
                iv0 = iv_pool.tile([1, 512], F32, tag="iv0",
                                   name=f"iv0{h}_{qc}")
                nc.vector.reciprocal_approx_fast(out=iv0, in_=sm0)
                bc = bc_pool.tile([64, 512], F32, tag="bc", name=f"bc{h}_{qc}")
                nc.gpsimd.partition_broadcast(bc, iv0)
                ct_i = h // 2
                if h % 2 == 0:
                    nc.vector.tensor_mul(
                        outT[ct_i][0:64, qc * 512:(qc + 1) * 512],
                        avs[0:64, :], bc)
                else:
                    ost = ost_pool.tile([64, 512], BF16, tag="ost",
                                        name=f"ost{h}_{qc}")
                    nc.vector.tensor_mul(ost, avs[0:64, :], bc)
                    nc.sync.dma_start(
                        out=outT[ct_i][64:128, qc * 512:(qc + 1) * 512],
                        in_=ost)

            wptt = None

            def emit_proj(tts):
                for tt in tts:
                    py = sc_ps.tile([128, 1024], F32, tag="sc",
                                    name=f"py{tt}")
                    for c in range(CT):
                        os_ = outT[c][:, tt * 128:(tt + 1) * 128]
                        nc.tensor.matmul(py[:, 0:512], os_, wptt[c][:, 0:512],
                                         start=(c == 0), stop=(c == CT - 1))
                        nc.tensor.matmul(py[:, 512:768], os_,
                                         wptt[c][:, 512:768],
                                         start=(c == 0), stop=(c == CT - 1))
                    ysb = y_pool.tile([128, C], F32, tag="y", name=f"y{tt}")
                    nc.vector.tensor_add(ysb, py[:, 0:768], bias_bc)
                    nc.sync.dma_start(out=y[tt * 128:(tt + 1) * 128, :],
                                      in_=ysb)

            # ---- software-pipelined schedule -------------------------------
            head_order = list(range(H))
            head_order[10], head_order[11] = head_order[11], head_order[10]
            chunks = [(h, qc) for h in head_order for qc in range(QC)]
            pv_slots = {0: (0, 1), 1: (2, 3), 2: (4, 5), 3: (6, 7)}

            sk_by_head = {head_order[0]: qk_project(head_order[0], wts0)}
            ets_store = {}
            av_done = 0

            for s, (h, qc) in enumerate(chunks):
                ets_store[s] = emit_sc_chunk(h, qc, sk_by_head[h])
                if s in pv_slots:
                    for tt in pv_slots[s]:
                        build_vaug(tt)
                # av chunks before qk: their DVE drain chain must precede the
                # qk casts in the FIFO (it releases the av PSUM bank)
                target = 0 if s < 3 else (s - 2 if s < 8 else s)
                while av_done < target:
                    ch, cq = chunks[av_done]
                    emit_av_chunk(ch, cq, ets_store.pop(av_done))
                    av_done += 1
                    if av_done == len(chunks) - 1:
                        emit_proj(range(0, 4))
                if qc == 0 and 2 * (s // 2) + 2 < len(chunks):
                    hn = chunks[s + 2][0]
                    sk_by_head[hn] = qk_project(hn, load_wqk(hn))
                if (h, qc) in ((4, 1), (5, 1)):
                    # prefetch output-projection weights in two half-bursts
                    # on the sync queue
                    if wptt is None:
                        wptt = []
                    for c in range(len(wptt), len(wptt) + 3):
                        t = w768_pool.tile([128, C], BF16, tag="w768",
                                           name=f"wpt{c}")
                        nc.sync.dma_start(out=t, in_=wpt[c, :, :])
                        wptt.append(t)

            while av_done < len(chunks):
                ch, cq = chunks[av_done]
                emit_av_chunk(ch, cq, ets_store.pop(av_done))
                av_done += 1
                if av_done == len(chunks) - 1:
                    emit_proj(range(0, 4))
            emit_proj(range(4, KT))

    nc.finalize()
    return nc


def _get_nc():
    global _NC_CACHE
    if _NC_CACHE is None:
        _NC_CACHE = _build()
    return _NC_CACHE


def _host_prep(x, Wqkv, Wproj, bproj, Aq, Bq, Av, Bv):
    """Fold LoRA + score scale into the weights; lay out and cast to bf16."""
    W = Wqkv.astype(np.float64)
    Wq = W[0:C].reshape(H, HD, C)
    Wk = W[C:2 * C].reshape(H, HD, C)
    Wv_ = W[2 * C:3 * C].reshape(H, HD, C)
    ABq = Aq.astype(np.float64) @ Bq.astype(np.float64)   # [HD, HD]
    ABv = Av.astype(np.float64) @ Bv.astype(np.float64)
    Wq = Wq + np.einsum('ed,hec->hdc', ABq, Wq)           # (I+AB).T @ Wq per head
    Wv_ = Wv_ + np.einsum('ed,hec->hdc', ABv, Wv_)
    Wq = Wq * SCALE                                       # fold softmax scale

    # wqk[h] = [K=c-rows(128), 6 c-tiles of (q_h cols(64) ++ k_h cols(64))]
    wqk = np.empty((H, 128, C), np.float32)
    for h in range(H):
        for c in range(CT):
            cs = slice(c * 128, (c + 1) * 128)
            wqk[h, :, c * 128:c * 128 + 64] = Wq[h][:, cs].T
            wqk[h, :, c * 128 + 64:(c + 1) * 128] = Wk[h][:, cs].T

    # wv[c] = [K=c-rows(128), all 768 v output features]
    WvT = Wv_.reshape(C, C).T.astype(np.float32)          # [c_in, v_out]
    wv = np.ascontiguousarray(WvT.reshape(CT, 128, C))

    # wpt[c] = Wproj.T c-tiles: [K=c(128), e(768)]
    WpT = Wproj.astype(np.float32).T                      # [c, e]
    wpt = np.ascontiguousarray(WpT.reshape(CT, 128, C))

    bf = ml_dtypes.bfloat16
    wqk = wqk.astype(bf)
    wv = wv.astype(bf)
    wpt = wpt.astype(bf)
    bias = bproj.astype(np.float32).reshape(1, C)

    per_core = []
    for b in range(B):
        xTb = np.ascontiguousarray(x[b].astype(np.float32).T.astype(bf))
        per_core.append({"xT": xTb, "wqk": wqk, "wv": wv, "wpt": wpt,
                         "bias": bias})
    return per_core


def kernel(x, Wqkv, Wproj, bproj, Aq, Bq, Av, Bv, _trace=False):
    x = np.asarray(x)
    in_maps = _host_prep(np.asarray(x), np.asarray(Wqkv), np.asarray(Wproj),
                         np.asarray(bproj), np.asarray(Aq), np.asarray(Bq),
                         np.asarray(Av), np.asarray(Bv))
    nc = _get_nc()
    res = run_bass_kernel_spmd(nc, in_maps, core_ids=list(range(N_CORES)),
                               trace=_trace)
    out = np.stack([res.results[b]["y"] for b in range(B)], axis=0)
    if _trace:
        kernel._last_result = res
    return out.astype(np.float32)
